# revision 52
# baseline (speedup 1.0000x reference)
"""Trainium2 Bass kernel for BasicMambaBlock (B=2, L=2048, d_model=1024).

Sharding: 8 cores = 2 batch groups x 4 TP shards.
Mamba half: d_inner TP-4 (512 ch/core), feature-major activations,
scan via tensor_tensor_scan in four L-quarters with chained state; the
y = sum_n C_n*h_n accumulation runs on the PE (identity-matmul into PSUM,
4 banks/quarter) so each quarter's out_proj matmuls + its ReduceScatter
chunk run inside the next quarter's scan shadow (DVE stays the only
critical resource at ~100% busy). All elementwise muls stay on DVE —
concurrent GpSimd tensor ops lock the shared SBUF port pair and slow
DVE perf-mode ops ~4x. The chunked RS hands each core an interleaved
[4 x 128]-row token set; host-side maps remap in/out accordingly.
FF half: token-sliced (512 tok/core) with FULL GEGLU weights streamed in
4 column chunks (3-D [128, k, cols] tiles); residual added on-chip.
All activations except the FF gelu are composed from Exp/Ln only (silu via
sigmoid(v) = exp(-ln(1+exp(-v)))) and the act-table chooser is steered to
the combined natural_log_exp set, so the scalar engine loads an activation
table ~3 times total instead of ~54. Small per-feature vectors are
host-pretiled into packed 2-D tiles (one DMA each; Sync triggers cost
~0.6us and serialize).
"""
import sys

sys.path.insert(0, "/opt/trn_rl_repo")

import numpy as np
import ml_dtypes
from contextlib import ExitStack

import concourse.bass as bass
import concourse.tile as tile
from concourse import bacc, mybir
from concourse.bass_utils import run_bass_kernel_spmd

FP32 = mybir.dt.float32
BF16 = mybir.dt.bfloat16
FP8 = mybir.dt.float8e4
ALU = mybir.AluOpType
ACTF = mybir.ActivationFunctionType
NPBF16 = ml_dtypes.bfloat16

import os

# Timing-only ablation: replace the two collectives with local DMA copies
# (results become WRONG; used to isolate HW collective cost).
NOCOLL = bool(int(os.environ.get("KB_NOCOLL", "0")))

DM = 1024          # d_model
DI = 2048          # d_inner (global)
DIS = DI // 4      # 512 per-core d_inner shard
NST = 16           # d_state
RNK = 64           # dt_rank
DC = 4             # conv width
FFI = 4096         # ff inner (global)
EPS = 1e-5
L_FULL = 2048
B_FULL = 2
LS = L_FULL // 4   # 512-token output slice per core


def _silu_div(nc, scr, out, x_ap, d_scr_tag, bias=None, nbias=None):
    """out = silu(x + b) = (x + b) * sigmoid(x + b), with
    sigmoid(v) = exp(-softplus(-v)) = exp(-ln(1 + exp(-v))).

    3 Act ops (Exp/Ln/Exp, all in the natural_log_exp table set used by the
    rest of the kernel) plus 1 DVE op. bias/nbias are (128,1) APs for +b
    and -b (None -> 0).
    """
    shape = [x_ap.shape[0], x_ap.free_size()]
    e = scr.tile(shape, BF16, tag=d_scr_tag + "_e")
    nc.scalar.activation(e[:], x_ap, ACTF.Exp, scale=-1.0,
                         bias=(nbias if nbias is not None else 0.0))
    l = scr.tile(shape, BF16, tag=d_scr_tag + "_l")
    nc.scalar.activation(l[:], e[:], ACTF.Ln, bias=1.0)
    s = scr.tile(shape, BF16, tag=d_scr_tag + "_s")
    nc.scalar.activation(s[:], l[:], ACTF.Exp, scale=-1.0)
    if bias is None:
        # tensor_tensor hits the DVE 2x perf mode; stt would run at 1x
        nc.vector.tensor_mul(out, x_ap, s[:])
    else:
        nc.vector.scalar_tensor_tensor(out, x_ap, bias, s[:],
                                       ALU.add, ALU.mult)


def _layer_norm_stage(nc, tc, ctx, src_tiles, n_tok_tiles, ident_sb, g_ap, b_ap,
                      hfm_pool, L, name, out_views=None, TCH=4):
    """Token-major LN on src_tiles (list of (128, DM) fp32 sbuf tiles) ->
    feature-major bf16 tiles (8 x (128, L)), with g/b applied per-partition
    after the transpose. Returns list of 8 hfm tiles.

    g_ap/b_ap are host-pretiled [128, 8] (column f = features f*128..):
    one DMA each instead of 8 tiny ones (Sync trigger time is ~0.6us per
    DMA and the triggers serialize on the Sync queue)."""
    stat = ctx.enter_context(tc.tile_pool(name=f"{name}_stat", bufs=4))
    scr = ctx.enter_context(tc.tile_pool(name=f"{name}_scr", bufs=2))
    nrm = ctx.enter_context(tc.tile_pool(name=f"{name}_nrm", bufs=6))
    gsb = ctx.enter_context(tc.tile_pool(name=f"{name}_gb", bufs=1))

    g_pack = gsb.tile([128, DM // 128], FP32, tag="gpack")
    nc.sync.dma_start(out=g_pack[:], in_=g_ap[:, :])
    b_pack = gsb.tile([128, DM // 128], FP32, tag="bpack")
    nc.sync.dma_start(out=b_pack[:], in_=b_ap[:, :])
    g_t = [g_pack[:, f:f + 1] for f in range(DM // 128)]
    b_t = [b_pack[:, f:f + 1] for f in range(DM // 128)]

    eps_t = gsb.tile([128, 1], FP32, tag="eps")
    nc.vector.memset(eps_t[:], EPS)

    # Group-of-TCH processing: stats packed into columns, batched mu/rstd
    # math on (128,TCH) slices, then normalize + transpose the group so the
    # first feature-major chunks are ready before the full LN finishes.
    nt_ = n_tok_tiles
    s1p = stat.tile([128, nt_], FP32, tag="s1p")
    s2p = stat.tile([128, nt_], FP32, tag="s2p")
    mu = stat.tile([128, nt_], FP32, tag="mu")
    musq = stat.tile([128, nt_], FP32, tag="musq")
    var = stat.tile([128, nt_], FP32, tag="var")
    lv = stat.tile([128, nt_], FP32, tag="lv")
    rstd = stat.tile([128, nt_], FP32, tag="rstd")
    psT = ctx.enter_context(tc.tile_pool(name=f"{name}_psT", bufs=2,
                                         space="PSUM"))
    if out_views is not None:
        hfm = out_views
    else:
        hfm = []
        for f in range(DM // 128):
            ht = hfm_pool.tile([128, L], BF16, tag="hfm")
            hfm.append(ht)

    for c in range(n_tok_tiles // TCH):
        g0 = c * TCH
        gs = slice(g0, g0 + TCH)
        for i in range(g0, g0 + TCH):
            xt = src_tiles[i]
            nc.vector.tensor_reduce(s1p[:, i:i + 1], xt[:],
                                    mybir.AxisListType.X, ALU.add)
            sq = scr.tile([128, DM], FP32, tag="sq")
            nc.scalar.activation(sq[:], xt[:], ACTF.Square,
                                 accum_out=s2p[:, i:i + 1])
        nc.vector.tensor_scalar_mul(mu[:, gs], s1p[:, gs], 1.0 / DM)
        nc.vector.tensor_mul(musq[:, gs], mu[:, gs], mu[:, gs])
        nc.vector.scalar_tensor_tensor(var[:, gs], s2p[:, gs], 1.0 / DM,
                                       musq[:, gs], ALU.mult, ALU.subtract)
        nc.scalar.activation(lv[:, gs], var[:, gs], ACTF.Ln, bias=eps_t[:])
        nc.scalar.activation(rstd[:, gs], lv[:, gs], ACTF.Exp, scale=-0.5)
        normed = []
        for i in range(g0, g0 + TCH):
            nt = nrm.tile([128, DM], BF16, tag="normed")
            nc.vector.tensor_scalar(nt[:], src_tiles[i][:], mu[:, i:i + 1],
                                    rstd[:, i:i + 1], ALU.subtract, ALU.mult)
            normed.append(nt)
        for f in range(DM // 128):
            pt = psT.tile([128, TCH * 128], BF16, tag="psT")
            for i in range(TCH):
                nc.tensor.transpose(pt[:, i * 128:(i + 1) * 128],
                                    normed[i][:, f * 128:(f + 1) * 128],
                                    ident_sb[:])
            nc.any.tensor_scalar(
                hfm[f][:, g0 * 128:(g0 + TCH) * 128], pt[:],
                g_t[f][:], b_t[f][:], ALU.mult, ALU.add)
    return hfm


def build_nc(L=L_FULL):
    n_tok = L // 128
    CH = min(512, L)
    n_ch = L // CH  # token chunks for matmul moving dim
    LH = L // 2     # scan half length
    n_res = LS // 128  # token tiles in this core's residual/FF slice

    nc = bacc.Bacc("TRN2", target_bir_lowering=False, debug=False,
                   num_devices=8)

    # The act-table chooser is first-match per function, so an Exp/Ln mix
    # alternates between the exp-only and ln-only sets (a ~1.3us table load
    # per switch). Steer both functions to the combined natural_log_exp set
    # (a real act_info set containing exp+ln) by hiding them in the others.
    from concourse.hw_specs import get_activation_tables
    try:
        tabs = get_activation_tables(nc.m.arch)
        for tname, tset in tabs.items():
            if tname != "natural_log_exp_and_others":
                tset.discard(ACTF.Exp)
                tset.discard(ACTF.Ln)
    except Exception:
        pass

    # ---- dram params ----
    def din(name, shape, dt=FP32):
        return nc.dram_tensor(name, shape, dt, kind="ExternalInput").ap()

    x_d = din("x", [L, DM], BF16)
    x_res_d = din("x_res", [LS, DM])               # this core's token slice
    # Small per-feature vectors are host-pretiled into packed 2-D tiles so
    # each loads with ONE DMA (Sync trigger time serializes otherwise).
    ln1_g = din("ln1_g", [128, DM // 128]); ln1_b = din("ln1_b", [128, DM // 128])
    ln2_g = din("ln2_g", [128, DM // 128]); ln2_b = din("ln2_b", [128, DM // 128])
    w_in_d = din("w_in", [DM, 2 * DIS], BF16)      # [xc cols | z cols]
    conv_w_d = din("conv_w", [DIS, DC])
    conv_b_d = din("conv_b", [128, 4])             # column d = channel tile d
    a_neg_d = din("a_neg", [DIS, NST])             # A = -exp(a_log) shard
    w_x_d = din("w_x", [DIS, RNK + 2 * NST], BF16)
    w_dt_d = din("w_dt", [RNK, DIS], BF16)
    b_dt_d = din("b_dt", [128, 4])
    d_skip_d = din("d_skip", [128, 4])
    w_out_d = din("w_out", [DIS, DM], BF16)
    w_ff1_d = din("w_ff1", [DM, 2 * FFI], BF16)    # FULL [a 4096 | g 4096]
    b_ff1_d = din("b_ff1", [2 * FFI, 1])           # FULL
    w_ff2_d = din("w_ff2", [FFI, DM], BF16)        # FULL
    ident_d = din("ident", [128, 128], BF16)

    out_d = nc.dram_tensor("out", [LS, DM], FP32, kind="ExternalOutput").ap()

    NCHK = 4               # FF column chunks
    FCH = FFI // NCHK      # 1024 a-cols + 1024 g-cols per chunk

    with tile.TileContext(nc) as tc, ExitStack() as octx:
        dram = octx.enter_context(tc.tile_pool(name="dram", bufs=1,
                                               space="DRAM"))
        const = octx.enter_context(tc.tile_pool(name="const", bufs=1))

        ident_sb = const.tile([128, 128], BF16, tag="ident")
        nc.sync.dma_start(out=ident_sb[:], in_=ident_d[:, :])

        # dram intermediates for collectives
        dbc_part = dram.tile([RNK + 2 * NST, L], BF16, tag="dbc_part")
        dbc_ar = dram.tile([RNK + 2 * NST, L], BF16, tag="dbc_ar")
        # ReduceScatter is chunked over 4 token blocks of 512 so each RS
        # overlaps the next block's out_proj matmuls. RS chunk k hands this
        # core rows [128*pos : 128*(pos+1)] of tokens [512k : 512(k+1)];
        # the host-side in/out maps are remapped to this interleaved
        # token ownership (LN2/FF are per-token, so compute is unaffected).
        m_part_c = [dram.tile([LS, DM], BF16, tag=f"m_part{k}",
                              name=f"m_part{k}") for k in range(4)]
        m_rs_c = [dram.tile([LS // 4, DM], BF16, tag=f"m_rs{k}",
                            name=f"m_rs{k}") for k in range(4)]

        groups = [[0, 1, 2, 3], [4, 5, 6, 7]]

        # Long-lived pools (phases 2-4) created first so shorter-lived
        # pools can pop in LIFO order before phase 5 reuses the space.
        pMain = octx.enter_context(ExitStack())
        wts = pMain.enter_context(tc.tile_pool(name="wts", bufs=1))
        sconst = pMain.enter_context(tc.tile_pool(name="sconst", bufs=1))
        act = pMain.enter_context(tc.tile_pool(name="act", bufs=1))
        zsp = pMain.enter_context(tc.tile_pool(name="zs", bufs=1))
        dtp = pMain.enter_context(tc.tile_pool(name="dtp", bufs=1))
        dtxp = pMain.enter_context(tc.tile_pool(name="dtx", bufs=1))
        dbcp = pMain.enter_context(tc.tile_pool(name="dbcp", bufs=1))

        # ================= Phase 1: LN1 -> h_fm =================
        p12 = pMain.enter_context(ExitStack())
        mm = p12.enter_context(tc.tile_pool(name="mm", bufs=4, space="PSUM"))
        hfm_pool = p12.enter_context(tc.tile_pool(name="hfm", bufs=8))
        with ExitStack() as p1, nc.named_scope("p1_ln1"):
            xload = p1.enter_context(tc.tile_pool(name="xload", bufs=6))
            xt_list = []
            for i in range(n_tok):
                xt = xload.tile([128, DM], BF16, tag="xt")
                nc.sync.dma_start(out=xt[:], in_=x_d[i * 128:(i + 1) * 128, :])
                xt_list.append(xt)
            hfm = _layer_norm_stage(nc, tc, p1, xt_list, n_tok, ident_sb,
                                    ln1_g, ln1_b, hfm_pool, L, "ln1")

        # ================= Phase 2: in_proj, conv, dbc, dt =================
        _sid2 = nc.enter_named_scope("p2_inproj", False)[0]
        w12 = p12.enter_context(tc.tile_pool(name="w12", bufs=1))
        # These early weight loads go out on the PE queue (idle at kernel
        # head) so their ~0.6us triggers don't serialize behind the x-tile
        # loads on the Sync queue.
        w_in_sb = []
        for k in range(8):
            t = w12.tile([128, 2 * DIS], BF16, tag=f"w_in{k}")
            nc.sync.dma_start(out=t[:], in_=w_in_d[k * 128:(k + 1) * 128, :])
            w_in_sb.append(t)
        wx_sb = []
        for k in range(4):
            t = w12.tile([128, RNK + 2 * NST], BF16, tag=f"wx{k}")
            nc.sync.dma_start(out=t[:], in_=w_x_d[k * 128:(k + 1) * 128, :])
            wx_sb.append(t)
        wdt_sb = w12.tile([RNK, DIS], BF16, tag="wdt")
        nc.sync.dma_start(out=wdt_sb[:], in_=w_dt_d[:, :])

        cw_sb, a_sb = [], []
        for d in range(4):
            r = slice(d * 128, (d + 1) * 128)
            t = sconst.tile([128, DC], FP32, tag=f"cw{d}")
            nc.sync.dma_start(out=t[:], in_=conv_w_d[r, :]); cw_sb.append(t)
            t = sconst.tile([128, NST], FP32, tag=f"a{d}")
            nc.sync.dma_start(out=t[:], in_=a_neg_d[r, :]); a_sb.append(t)
        cb_pack = sconst.tile([128, 4], FP32, tag="cb_pack")
        nc.sync.dma_start(out=cb_pack[:], in_=conv_b_d[:, :])
        cbn_pack = sconst.tile([128, 4], FP32, tag="cbn_pack")
        nc.vector.tensor_scalar_mul(cbn_pack[:], cb_pack[:], -1.0)
        bdt_pack = sconst.tile([128, 4], FP32, tag="bdt_pack")
        nc.sync.dma_start(out=bdt_pack[:], in_=b_dt_d[:, :])
        dskip_pack = sconst.tile([128, 4], FP32, tag="dskip_pack")
        nc.sync.dma_start(out=dskip_pack[:], in_=d_skip_d[:, :])
        cb_sb = [cb_pack[:, d:d + 1] for d in range(4)]
        cbn_sb = [cbn_pack[:, d:d + 1] for d in range(4)]
        bdt_sb = [bdt_pack[:, d:d + 1] for d in range(4)]
        dskip_sb = [dskip_pack[:, d:d + 1] for d in range(4)]

        xc_pad = []
        for d in range(4):
            t = act.tile([128, L + 3], BF16, tag=f"xcp{d}")
            nc.vector.memset(t[:, 0:3], 0.0)
            xc_pad.append(t)
        zraw = p12.enter_context(tc.tile_pool(name="zraw", bufs=1))
        z_sb = [zraw.tile([128, L], BF16, tag=f"z{d}", name=f"z{d}")
                for d in range(4)]

        # in_proj xc columns only (z columns are deferred into the
        # dbc-AllReduce window below)
        def inproj_f(f):
            for c in range(n_ch):
                ps = mm.tile([128, CH], FP32, tag="mm", name="ps")
                for k in range(8):
                    nc.tensor.matmul(
                        ps[:], w_in_sb[k][:, f * 128:(f + 1) * 128],
                        hfm[k][:, c * CH:(c + 1) * CH],
                        start=(k == 0), stop=(k == 7))
                if f < 4:
                    nc.any.tensor_copy(
                        xc_pad[f][:, 3 + c * CH: 3 + (c + 1) * CH], ps[:])
                else:
                    # Scalar-pinned: these run while DVE works the conv/silu
                    # chain on the critical path.
                    nc.scalar.copy(
                        z_sb[f - 4][:, c * CH:(c + 1) * CH], ps[:])

        for f in range(4):
            inproj_f(f)

        # conv + silu (writes silu'd xc back into xc_pad[:, 3:3+L]).
        # tensor_scalar runs in the DVE 2x/4x perf modes while
        # scalar_tensor_tensor is stuck at 1x, so the tap muls go through
        # separate tensor_scalar ops + 2x adds instead of an stt chain.
        # conv+silu run per L-half so the dbc matmuls (which consume 512-col
        # chunks) start on half-0 while half-1 is still in flight.
        cacc = p12.enter_context(tc.tile_pool(name="cacc", bufs=2))
        for d in range(4):
            acc = cacc.tile([128, L], BF16, tag="cacc")
            # Both halves' conv taps are emitted before either silu: the silu
            # overwrites xc_pad in place, and half-1's taps read the raw
            # boundary columns half-0's silu would clobber.
            for h in range(2):
                hs = slice(h * LH, (h + 1) * LH)
                nc.vector.tensor_scalar_mul(acc[:, hs],
                                            xc_pad[d][:, h * LH:h * LH + LH],
                                            cw_sb[d][:, 0:1])
                for j in range(1, DC):
                    tj = cacc.tile([128, LH], BF16, tag="ctap")
                    nc.vector.tensor_scalar_mul(
                        tj[:], xc_pad[d][:, j + h * LH:j + h * LH + LH],
                        cw_sb[d][:, j:j + 1])
                    if j == DC - 1:
                        # conv bias via a 4x-mode tensor_scalar add, then a
                        # 2x add (the stt equivalent runs at 1x)
                        nc.vector.tensor_scalar_add(tj[:], tj[:],
                                                    cb_sb[d][:])
                    nc.vector.tensor_add(acc[:, hs], acc[:, hs], tj[:])
            for h in range(2):
                hs = slice(h * LH, (h + 1) * LH)
                _silu_div(nc, cacc,
                          xc_pad[d][:, 3 + h * LH:3 + h * LH + LH],
                          acc[:, hs], "sil")

        # dbc partial + AllReduce
        nc.leave_named_scope("p2_inproj", _sid2, False)
        _sid2b = nc.enter_named_scope("p2b_dbc_ar", False)[0]
        dbc_sb = dbcp.tile([RNK + 2 * NST, L], BF16, tag="dbc")
        for c in range(n_ch):
            ps = mm.tile([RNK + 2 * NST, CH], FP32, tag="mm")
            for k in range(4):
                nc.tensor.matmul(ps[:], wx_sb[k][:],
                                 xc_pad[k][:, 3 + c * CH:3 + (c + 1) * CH],
                                 start=(k == 0), stop=(k == 3))
            nc.any.tensor_copy(dbc_sb[:, c * CH:(c + 1) * CH], ps[:])
        nc.sync.dma_start(out=dbc_part[:], in_=dbc_sb[:])
        if NOCOLL:
            nc.sync.dma_start(out=dbc_ar[:], in_=dbc_part[:])
        else:
            nc.gpsimd.collective_compute(
                "AllReduce", ALU.add, replica_groups=groups,
                ins=[dbc_part.opt()], outs=[dbc_ar.opt()])

        # ---- overlapped with the AllReduce: z in_proj, z-silu, prefetches
        for f in range(4, 8):
            inproj_f(f)
        zs_sb = []
        for d in range(4):
            zs = zsp.tile([128, L], BF16, tag=f"zs{d}")
            _silu_div(nc, cacc, zs[:], z_sb[d][:], "sil")
            zs_sb.append(zs)
        wout_sb = []
        for k in range(4):
            t = wts.tile([128, DM], BF16, tag=f"wout{k}")
            nc.sync.dma_start(out=t[:], in_=w_out_d[k * 128:(k + 1) * 128, :])
            wout_sb.append(t)

        nc.sync.dma_start(out=dbc_sb[:], in_=dbc_ar[:])
        dtlo = dbc_sb[0:RNK, :]
        nc.leave_named_scope("p2b_dbc_ar", _sid2b, False)
        _sid2c = nc.enter_named_scope("p2c_dt", False)[0]

        # dt = softplus(dt_lo @ w_dt + b_dt) = Ln(Exp(u + b_dt) + 1)
        # Half-0 chunks (0,1) are fully computed here so the scan can start;
        # half-1 chunks only run their matmul (PSUM dies with p12) and spill
        # the raw pre-activation, with softplus+dtx deferred into the scan's
        # half-0 window where the Act engine has slack.
        spscr = p12.enter_context(tc.tile_pool(name="spscr", bufs=3))
        dt_sb = [dtp.tile([128, L], BF16, tag=f"dt{d}", name=f"dt{d}")
                 for d in range(4)]
        dtx_sb = [dtxp.tile([128, L], BF16, tag=f"dtx{d}", name=f"dtx{d}")
                  for d in range(4)]
        uraw = [dtp.tile([128, LH], BF16, tag=f"u{d}", name=f"u{d}")
                for d in range(4)]
        for c in range(n_ch):
            cs = slice(c * CH, (c + 1) * CH)
            for d in range(4):
                ps = mm.tile([128, CH], FP32, tag="mm", name="ps")
                nc.tensor.matmul(ps[:], wdt_sb[:, d * 128:(d + 1) * 128],
                                 dtlo[:, cs].opt(),
                                 start=True, stop=True)
                if c < 2:
                    e = spscr.tile([128, CH], FP32, tag="sp_e")
                    nc.scalar.activation(e[:], ps[:], ACTF.Exp,
                                         bias=bdt_sb[d][:])
                    nc.scalar.activation(dt_sb[d][:, cs], e[:], ACTF.Ln,
                                         bias=1.0)
                    nc.vector.tensor_mul(
                        dtx_sb[d][:, cs], dt_sb[d][:, cs],
                        xc_pad[d][:, 3 + c * CH:3 + (c + 1) * CH])
                else:
                    nc.any.tensor_copy(
                        uraw[d][:, (c - 2) * CH:(c - 1) * CH], ps[:])
        p12.close()

        def emit_dt_half1(spool):
            for c in (2, 3):
                cs = slice(c * CH, (c + 1) * CH)
                us = slice((c - 2) * CH, (c - 1) * CH)
                for d in range(4):
                    e = spool.tile([128, CH], FP32, tag="sp_e1", name="e1")
                    nc.scalar.activation(e[:], uraw[d][:, us], ACTF.Exp,
                                         bias=bdt_sb[d][:])
                    nc.scalar.activation(dt_sb[d][:, cs], e[:], ACTF.Ln,
                                         bias=1.0)
                    nc.vector.tensor_mul(
                        dtx_sb[d][:, cs], dt_sb[d][:, cs],
                        xc_pad[d][:, 3 + c * CH:3 + (c + 1) * CH])

        nc.leave_named_scope("p2c_dt", _sid2c, False)
        _sid3 = nc.enter_named_scope("p3_scan", False)[0]

        # ========= Phase 3+4: scan + gate + out_proj + RS, fused =========
        # Four L-quarters instead of two halves: the y PSUM accumulation for
        # a quarter needs only 4 banks (4 d-tiles x [128,512] fp32), leaving
        # 2 banks for out_proj, so each quarter's out_proj matmuls and its
        # ReduceScatter chunk run inside the next quarter's scan shadow
        # instead of serially after the whole scan.
        LQ = L // 4
        p34 = pMain.enter_context(ExitStack())
        gp = p34.enter_context(tc.tile_pool(name="gated", bufs=1))
        ypsum = p34.enter_context(tc.tile_pool(name="ypsum", bufs=1,
                                               space="PSUM"))
        mmo = p34.enter_context(tc.tile_pool(name="mmo", bufs=2, space="PSUM"))
        mp_pool = p34.enter_context(tc.tile_pool(name="mp", bufs=3))
        sc = p34.enter_context(tc.tile_pool(name="scan", bufs=2))
        bcp = p34.enter_context(tc.tile_pool(name="bcast", bufs=2))
        hlast = [sconst.tile([128, NST], FP32, tag=f"hl{d}", name=f"hl{d}")
                 for d in range(4)]
        gated_sb = [gp.tile([128, L], BF16, tag=f"g{d}", name=f"g{d}")
                    for d in range(4)]

        for q in range(4):
            off = q * LQ
            y_ps = [ypsum.tile([128, LQ], FP32, tag=f"yps{d}", name=f"yps{d}")
                    for d in range(4)]
            for n0 in range(0, NST, 8):
                # States are processed in quads: the dB and hC multiplies for
                # (n0..n0+3) fuse into single [128, 4, LQ] DVE ops (2x perf
                # mode; per-instruction overhead amortizes over 2048 cols),
                # with dtx broadcast stride-0 across the quad dim. Scans stay
                # per-n (serial recurrence).
                bcB = bcp.tile([128, 8, LQ], BF16, tag="bcB")
                bcC = bcp.tile([128, 8, LQ], BF16, tag="bcC")
                for j in range(8):
                    nc.sync.dma_start(
                        out=bcB[:, j, :],
                        in_=dbc_ar[RNK + n0 + j:RNK + n0 + j + 1,
                                   off:off + LQ].broadcast_to([128, LQ]))
                    nc.sync.dma_start(
                        out=bcC[:, j, :],
                        in_=dbc_ar[RNK + NST + n0 + j:RNK + NST + n0 + j + 1,
                                   off:off + LQ].broadcast_to([128, LQ]))
                for d in range(4):
                    dA = sc.tile([128, 8, LQ], BF16, tag="dA")
                    for j in range(8):
                        nc.scalar.activation(
                            dA[:, j, :], dt_sb[d][:, off:off + LQ],
                            ACTF.Exp, scale=a_sb[d][:, n0 + j:n0 + j + 1])
                    dB = sc.tile([128, 8, LQ], BF16, tag="dB")
                    nc.vector.tensor_mul(
                        dB[:, :, :],
                        dtx_sb[d][:, off:off + LQ].unsqueeze(1)
                        .broadcast_to([128, 8, LQ]),
                        bcB[:, :, :])
                    for j in range(8):
                        n = n0 + j
                        # h state scan (in-place over dB), fp32 internal state
                        init = 0.0 if q == 0 else hlast[d][:, n:n + 1]
                        nc.vector.tensor_tensor_scan(
                            dB[:, j, :], dA[:, j, :], dB[:, j, :], init,
                            ALU.mult, ALU.add)
                        if q < 3:
                            nc.scalar.activation(
                                hlast[d][:, n:n + 1],
                                dB[:, j, LQ - 1:LQ], ACTF.Identity)
                    # hC into dA tile (reuse), then accumulate on PE.
                    # All hC muls stay on DVE: a concurrent GpSimd tensor op
                    # locks the shared SBUF port pair and quadruples the DVE
                    # op latency (602ns -> 2460ns measured), so offloading to
                    # Pool is strictly counterproductive.
                    nc.vector.tensor_mul(dA[:, :, :], dB[:, :, :], bcC[:, :, :])
                    for j in range(8):
                        nc.tensor.matmul(
                            y_ps[d][:], ident_sb[:], dA[:, j, :],
                            start=(n0 + j == 0), stop=(n0 + j == NST - 1))
            if q == 1:
                emit_dt_half1(sc)
            # gate: gated = (y + d_skip*xc) * silu(z)
            for d in range(4):
                tmp = sc.tile([128, LQ], FP32, tag="gtmp")
                nc.vector.scalar_tensor_tensor(
                    tmp[:], xc_pad[d][:, 3 + off:3 + off + LQ],
                    dskip_sb[d][:], y_ps[d][:], ALU.mult, ALU.add)
                nc.vector.tensor_mul(gated_sb[d][:, off:off + LQ], tmp[:],
                                     zs_sb[d][:, off:off + LQ])
            # out_proj for this quarter's 4 token tiles + its RS chunk
            for i in range(4 * q, 4 * q + 4):
                mp = mp_pool.tile([128, DM], BF16, tag="mp")
                for nchk in range(2):
                    ps = mmo.tile([128, 512], FP32, tag="mmo")
                    for k in range(4):
                        nc.tensor.matmul(
                            ps[:], gated_sb[k][:, i * 128:(i + 1) * 128],
                            wout_sb[k][:, nchk * 512:(nchk + 1) * 512],
                            start=(k == 0), stop=(k == 3))
                    # Pinned to Scalar: DVE is saturated during the scan.
                    nc.scalar.copy(mp[:, nchk * 512:(nchk + 1) * 512], ps[:])
                nc.sync.dma_start(
                    out=m_part_c[q][(i % 4) * 128:(i % 4 + 1) * 128, :],
                    in_=mp[:])
            if NOCOLL:
                nc.sync.dma_start(out=m_rs_c[q][:],
                                  in_=m_part_c[q][0:LS // 4, :])
            else:
                nc.gpsimd.collective_compute(
                    "ReduceScatter", ALU.add, replica_groups=groups,
                    ins=[m_part_c[q].opt()], outs=[m_rs_c[q].opt()])

        nc.leave_named_scope("p3_scan", _sid3, False)
        _sid4 = nc.enter_named_scope("p4_outproj", False)[0]
        p34.close()
        pMain.close()

        # ---- overlapped with the ReduceScatter: FF chunk-0 weight loads ----
        wf = octx.enter_context(tc.tile_pool(name="wf", bufs=2))
        wf2 = octx.enter_context(tc.tile_pool(name="wf2", bufs=2))
        bfp = octx.enter_context(tc.tile_pool(name="bfp", bufs=1))

        def load_ff_chunk(c):
            # fp8 weights in 3-D [128, k_subtile, cols] layout so matmuls can
            # take [:, 2k:2k+2, :] DoubleRow slices (K=256 per matmul).
            a0, g0 = c * FCH, FFI + c * FCH
            w1a = wf.tile([128, 8, FCH], BF16, tag="w1a", name="w1a")
            w1g = wf.tile([128, 8, FCH], BF16, tag="w1g", name="w1g")
            for k in range(8):
                nc.sync.dma_start(
                    out=w1a[:, k, :],
                    in_=w_ff1_d[k * 128:(k + 1) * 128, a0:a0 + FCH])
                nc.sync.dma_start(
                    out=w1g[:, k, :],
                    in_=w_ff1_d[k * 128:(k + 1) * 128, g0:g0 + FCH])
            w2 = wf2.tile([128, 8, DM], BF16, tag="w2", name="w2")
            for k in range(8):
                nc.sync.dma_start(
                    out=w2[:, k, :],
                    in_=w_ff2_d[c * FCH + k * 128:c * FCH + (k + 1) * 128, :])
            ba = [bfp.tile([128, 1], FP32, tag=f"ba{c}_{j}", name=f"ba{c}_{j}")
                  for j in range(8)]
            bg = [bfp.tile([128, 1], FP32, tag=f"bg{c}_{j}", name=f"bg{c}_{j}")
                  for j in range(8)]
            for j in range(8):
                nc.sync.dma_start(
                    out=ba[j][:],
                    in_=b_ff1_d[a0 + j * 128:a0 + (j + 1) * 128, :])
                nc.sync.dma_start(
                    out=bg[j][:],
                    in_=b_ff1_d[g0 + j * 128:g0 + (j + 1) * 128, :])
            return w1a, w1g, w2, ba, bg

        ff_chunk = load_ff_chunk(0)

        # ================= Phase 5: residual + LN2 (own L/4 slice) ==========
        # h2 is written straight to fp8 in 3-D [128, k, LS] layout for the
        # DoubleRow ff1 matmuls.
        h2fm_pool = octx.enter_context(tc.tile_pool(name="h2fm", bufs=1))
        h2t = h2fm_pool.tile([128, 8, LS], BF16, tag="h2t", name="h2t")
        h2_views = [h2t[:, f, :] for f in range(8)]
        x2p = octx.enter_context(tc.tile_pool(name="x2", bufs=1))
        xres_p = octx.enter_context(tc.tile_pool(name="xres", bufs=1))
        xres_sb = []
        for i in range(n_res):
            t = xres_p.tile([128, DM], FP32, tag=f"xres{i}", name=f"xres{i}")
            nc.sync.dma_start(out=t[:], in_=x_res_d[i * 128:(i + 1) * 128, :])
            xres_sb.append(t)
        x2_list = []
        with ExitStack() as p5, nc.named_scope("p5_ln2"):
            ld = p5.enter_context(tc.tile_pool(name="ld5", bufs=3))
            for i in range(n_res):
                mt = ld.tile([128, DM], BF16, tag="mr")
                nc.sync.dma_start(out=mt[:], in_=m_rs_c[i][:, :])
                x2 = x2p.tile([128, DM], FP32, tag=f"x2_{i}")
                nc.vector.tensor_add(x2[:], xres_sb[i][:], mt[:])
                x2_list.append(x2)
            # TCH=2: group {0,1} only needs RS chunks 0-1, so most of LN2
            # runs inside the scan shadow; only the tile-3 group trails RS_3.
            h2fm = _layer_norm_stage(nc, tc, p5, x2_list, n_res, ident_sb,
                                     ln2_g, ln2_b, h2fm_pool, LS, "ln2",
                                     out_views=h2_views, TCH=2)

        # ================= Phase 6: FF (full width, 4 chunks) ===============
        with ExitStack() as p6, nc.named_scope("p6_ff"):
            mma6 = p6.enter_context(tc.tile_pool(name="mma6", bufs=2,
                                                 space="PSUM"))
            mm6 = p6.enter_context(tc.tile_pool(name="mm6", bufs=2,
                                                space="PSUM"))
            pso_pool = p6.enter_context(tc.tile_pool(name="pso", bufs=2,
                                                     space="PSUM"))
            agp = p6.enter_context(tc.tile_pool(name="ag", bufs=2))
            tmp6 = p6.enter_context(tc.tile_pool(name="tmp6", bufs=4))
            acc_p = p6.enter_context(tc.tile_pool(name="ffacc", bufs=1))
            out_acc = [acc_p.tile([128, DM], FP32, tag=f"oacc{i}", name=f"oacc{i}")
                       for i in range(n_res)]

            for c in range(NCHK):
                w1a, w1g, w2, ba, bg = ff_chunk
                if c + 1 < NCHK:
                    next_chunk = load_ff_chunk(c + 1)
                ag_t = agp.tile([128, 8, LS], BF16, tag="ag")
                for sub in range(FCH // 128):
                    psA = mma6.tile([128, LS], FP32, tag="mma6")
                    psG = mm6.tile([128, LS], FP32, tag="mm6")
                    for k in range(8):
                        nc.tensor.matmul(
                            psA[:], w1a[:, k, sub * 128:(sub + 1) * 128],
                            h2t[:, k, :], start=(k == 0), stop=(k == 7))
                    for k in range(8):
                        nc.tensor.matmul(
                            psG[:], w1g[:, k, sub * 128:(sub + 1) * 128],
                            h2t[:, k, :], start=(k == 0), stop=(k == 7))
                    aa = tmp6.tile([128, LS], BF16, tag="aa")
                    nc.scalar.activation(aa[:], psA[:], ACTF.Identity,
                                         bias=ba[sub][:])
                    gg = tmp6.tile([128, LS], BF16, tag="gg")
                    nc.scalar.activation(gg[:], psG[:], ACTF.Gelu_apprx_tanh,
                                         bias=bg[sub][:])
                    nc.vector.tensor_mul(ag_t[:, sub, :], aa[:], gg[:])
                # ff2: token-major, accumulate chunks in SBUF fp32
                for tt in range(n_res):
                    pso = pso_pool.tile([128, DM], FP32, tag="pso")
                    for q in range(DM // 512):
                        for k in range(8):
                            nc.tensor.matmul(
                                pso[:, q * 512:(q + 1) * 512],
                                ag_t[:, k, tt * 128:(tt + 1) * 128],
                                w2[:, k, q * 512:(q + 1) * 512],
                                start=(k == 0), stop=(k == 7))
                    if c == 0:
                        nc.any.tensor_copy(out_acc[tt][:], pso[:])
                    else:
                        nc.vector.tensor_add(out_acc[tt][:], out_acc[tt][:],
                                             pso[:])
                if c + 1 < NCHK:
                    ff_chunk = next_chunk

            # final: out = x2 + ff
            outp = p6.enter_context(tc.tile_pool(name="outp", bufs=2))
            for i in range(n_res):
                ot = outp.tile([128, DM], FP32, tag="ot")
                nc.vector.tensor_add(ot[:], x2_list[i][:], out_acc[i][:])
                nc.sync.dma_start(out=out_d[i * 128:(i + 1) * 128, :],
                                  in_=ot[:])
        nc.leave_named_scope("p4_outproj", _sid4, False)
    nc.compile()
    return nc


_NC_CACHE = {}


def _get_nc(L=L_FULL):
    if L not in _NC_CACHE:
        _NC_CACHE[L] = build_nc(L)
    return _NC_CACHE[L]


def make_in_maps(x, ln1_g, ln1_b, w_in, conv_w, conv_b, w_x, w_dt, b_dt,
                 a_log, d_skip, w_out, ln2_g, ln2_b, w_ff1, b_ff1, w_ff2,
                 b_ff2):
    x = np.asarray(x, np.float32)
    f32 = lambda a: np.ascontiguousarray(np.asarray(a, np.float32))
    bf = lambda a: np.ascontiguousarray(np.asarray(a, np.float32)).astype(NPBF16)
    # pack a per-feature vector [n*128] -> [128, n] (column j = slice j)
    pack = lambda a, n: f32(np.asarray(a, np.float32).reshape(n, 128).T)
    ident = np.eye(128, dtype=np.float32).astype(NPBF16)
    a_neg = -np.exp(np.asarray(a_log, np.float32))
    w_ff1_b = bf(w_ff1)
    w_ff2_b = bf(w_ff2)
    b_ff1_c = f32(b_ff1).reshape(2 * FFI, 1)
    in_maps = []
    for c in range(8):
        b, s = c // 4, c % 4
        ds = slice(s * DIS, (s + 1) * DIS)
        # Chunked-RS token ownership: this core owns rows
        # [512k + 128s : 512k + 128(s+1)] for k = 0..3.
        own = np.concatenate(
            [x[b][512 * k + 128 * s: 512 * k + 128 * (s + 1)]
             for k in range(4)], axis=0)
        in_maps.append(dict(
            x=bf(x[b]),
            x_res=f32(own),
            ln1_g=pack(ln1_g, 8), ln1_b=pack(ln1_b, 8),
            ln2_g=pack(ln2_g, 8), ln2_b=pack(ln2_b, 8),
            w_in=bf(np.concatenate(
                [w_in[:, s * DIS:(s + 1) * DIS],
                 w_in[:, DI + s * DIS:DI + (s + 1) * DIS]], axis=1)),
            conv_w=f32(conv_w[ds]), conv_b=pack(np.asarray(conv_b)[ds], 4),
            a_neg=f32(a_neg[ds]),
            w_x=bf(w_x[ds]), w_dt=bf(w_dt[:, ds]),
            b_dt=pack(np.asarray(b_dt)[ds], 4),
            d_skip=pack(np.asarray(d_skip)[ds], 4),
            w_out=bf(w_out[ds]),
            w_ff1=w_ff1_b, b_ff1=b_ff1_c, w_ff2=w_ff2_b,
            ident=ident,
        ))
    return in_maps


def combine_outputs(results, b_ff2, L=L_FULL):
    out = np.zeros((B_FULL, L, DM), np.float32)
    bff2 = np.asarray(b_ff2, np.float32)
    for b in range(B_FULL):
        for s in range(4):
            res = results[4 * b + s]["out"].astype(np.float32) + bff2[None, :]
            for k in range(4):
                out[b, 512 * k + 128 * s: 512 * k + 128 * (s + 1)] = (
                    res[128 * k: 128 * (k + 1)])
    return out


def kernel(**inputs):
    nc = _get_nc(L_FULL)
    in_maps = make_in_maps(
        inputs["x"], inputs["ln1_g"], inputs["ln1_b"], inputs["w_in"],
        inputs["conv_w"], inputs["conv_b"], inputs["w_x"], inputs["w_dt"],
        inputs["b_dt"], inputs["a_log"], inputs["d_skip"], inputs["w_out"],
        inputs["ln2_g"], inputs["ln2_b"], inputs["w_ff1"], inputs["b_ff1"],
        inputs["w_ff2"], inputs["b_ff2"])
    res = run_bass_kernel_spmd(nc, in_maps, core_ids=list(range(8)))
    return combine_outputs(res.results, inputs["b_ff2"], L_FULL)



# revision 53
# speedup vs baseline: 1.0178x; 1.0178x over previous
"""Trainium2 Bass kernel for BasicMambaBlock (B=2, L=2048, d_model=1024).

Sharding: 8 cores = 2 batch groups x 4 TP shards.
Mamba half: d_inner TP-4 (512 ch/core), feature-major activations,
scan via tensor_tensor_scan in four L-quarters with chained state; the
y = sum_n C_n*h_n accumulation runs on the PE (identity-matmul into PSUM,
4 banks/quarter) so each quarter's out_proj matmuls + its ReduceScatter
chunk run inside the next quarter's scan shadow (DVE stays the only
critical resource at ~100% busy). All elementwise muls stay on DVE —
concurrent GpSimd tensor ops lock the shared SBUF port pair and slow
DVE perf-mode ops ~4x. The chunked RS hands each core an interleaved
[4 x 128]-row token set; host-side maps remap in/out accordingly.
FF half: token-sliced (512 tok/core) with FULL GEGLU weights streamed in
4 column chunks (3-D [128, k, cols] tiles); residual added on-chip.
All activations except the FF gelu are composed from Exp/Ln only (silu via
sigmoid(v) = exp(-ln(1+exp(-v)))) and the act-table chooser is steered to
the combined natural_log_exp set, so the scalar engine loads an activation
table ~3 times total instead of ~54. Small per-feature vectors are
host-pretiled into packed 2-D tiles (one DMA each; Sync triggers cost
~0.6us and serialize).
"""
import sys

sys.path.insert(0, "/opt/trn_rl_repo")

import numpy as np
import ml_dtypes
from contextlib import ExitStack

import concourse.bass as bass
import concourse.tile as tile
from concourse import bacc, mybir
from concourse.bass_utils import run_bass_kernel_spmd

FP32 = mybir.dt.float32
BF16 = mybir.dt.bfloat16
FP8 = mybir.dt.float8e4
ALU = mybir.AluOpType
ACTF = mybir.ActivationFunctionType
NPBF16 = ml_dtypes.bfloat16

import os

# Timing-only ablation: replace the two collectives with local DMA copies
# (results become WRONG; used to isolate HW collective cost).
NOCOLL = bool(int(os.environ.get("KB_NOCOLL", "0")))

DM = 1024          # d_model
DI = 2048          # d_inner (global)
DIS = DI // 4      # 512 per-core d_inner shard
NST = 16           # d_state
RNK = 64           # dt_rank
DC = 4             # conv width
FFI = 4096         # ff inner (global)
EPS = 1e-5
L_FULL = 2048
B_FULL = 2
LS = L_FULL // 4   # 512-token output slice per core


def _silu_div(nc, scr, out, x_ap, d_scr_tag, bias=None, nbias=None):
    """out = silu(x + b) = (x + b) * sigmoid(x + b), with
    sigmoid(v) = exp(-softplus(-v)) = exp(-ln(1 + exp(-v))).

    3 Act ops (Exp/Ln/Exp, all in the natural_log_exp table set used by the
    rest of the kernel) plus 1 DVE op. bias/nbias are (128,1) APs for +b
    and -b (None -> 0).
    """
    shape = [x_ap.shape[0], x_ap.free_size()]
    e = scr.tile(shape, BF16, tag=d_scr_tag + "_e")
    nc.scalar.activation(e[:], x_ap, ACTF.Exp, scale=-1.0,
                         bias=(nbias if nbias is not None else 0.0))
    l = scr.tile(shape, BF16, tag=d_scr_tag + "_l")
    nc.scalar.activation(l[:], e[:], ACTF.Ln, bias=1.0)
    s = scr.tile(shape, BF16, tag=d_scr_tag + "_s")
    nc.scalar.activation(s[:], l[:], ACTF.Exp, scale=-1.0)
    if bias is None:
        # tensor_tensor hits the DVE 2x perf mode; stt would run at 1x
        nc.vector.tensor_mul(out, x_ap, s[:])
    else:
        nc.vector.scalar_tensor_tensor(out, x_ap, bias, s[:],
                                       ALU.add, ALU.mult)


def _layer_norm_stage(nc, tc, ctx, src_tiles, n_tok_tiles, ident_sb, g_ap, b_ap,
                      hfm_pool, L, name, out_views=None, TCH=4):
    """Token-major LN on src_tiles (list of (128, DM) fp32 sbuf tiles) ->
    feature-major bf16 tiles (8 x (128, L)), with g/b applied per-partition
    after the transpose. Returns list of 8 hfm tiles.

    g_ap/b_ap are host-pretiled [128, 8] (column f = features f*128..):
    one DMA each instead of 8 tiny ones (Sync trigger time is ~0.6us per
    DMA and the triggers serialize on the Sync queue)."""
    stat = ctx.enter_context(tc.tile_pool(name=f"{name}_stat", bufs=4))
    scr = ctx.enter_context(tc.tile_pool(name=f"{name}_scr", bufs=2))
    nrm = ctx.enter_context(tc.tile_pool(name=f"{name}_nrm", bufs=6))
    gsb = ctx.enter_context(tc.tile_pool(name=f"{name}_gb", bufs=1))

    g_pack = gsb.tile([128, DM // 128], FP32, tag="gpack")
    nc.sync.dma_start(out=g_pack[:], in_=g_ap[:, :])
    b_pack = gsb.tile([128, DM // 128], FP32, tag="bpack")
    nc.sync.dma_start(out=b_pack[:], in_=b_ap[:, :])
    g_t = [g_pack[:, f:f + 1] for f in range(DM // 128)]
    b_t = [b_pack[:, f:f + 1] for f in range(DM // 128)]

    eps_t = gsb.tile([128, 1], FP32, tag="eps")
    nc.vector.memset(eps_t[:], EPS)

    # Group-of-TCH processing: stats packed into columns, batched mu/rstd
    # math on (128,TCH) slices, then normalize + transpose the group so the
    # first feature-major chunks are ready before the full LN finishes.
    nt_ = n_tok_tiles
    s1p = stat.tile([128, nt_], FP32, tag="s1p")
    s2p = stat.tile([128, nt_], FP32, tag="s2p")
    mu = stat.tile([128, nt_], FP32, tag="mu")
    musq = stat.tile([128, nt_], FP32, tag="musq")
    var = stat.tile([128, nt_], FP32, tag="var")
    lv = stat.tile([128, nt_], FP32, tag="lv")
    rstd = stat.tile([128, nt_], FP32, tag="rstd")
    psT = ctx.enter_context(tc.tile_pool(name=f"{name}_psT", bufs=2,
                                         space="PSUM"))
    if out_views is not None:
        hfm = out_views
    else:
        hfm = []
        for f in range(DM // 128):
            ht = hfm_pool.tile([128, L], BF16, tag="hfm")
            hfm.append(ht)

    for c in range(n_tok_tiles // TCH):
        g0 = c * TCH
        gs = slice(g0, g0 + TCH)
        for i in range(g0, g0 + TCH):
            xt = src_tiles[i]
            nc.vector.tensor_reduce(s1p[:, i:i + 1], xt[:],
                                    mybir.AxisListType.X, ALU.add)
            sq = scr.tile([128, DM], FP32, tag="sq")
            nc.scalar.activation(sq[:], xt[:], ACTF.Square,
                                 accum_out=s2p[:, i:i + 1])
        nc.vector.tensor_scalar_mul(mu[:, gs], s1p[:, gs], 1.0 / DM)
        nc.vector.tensor_mul(musq[:, gs], mu[:, gs], mu[:, gs])
        nc.vector.scalar_tensor_tensor(var[:, gs], s2p[:, gs], 1.0 / DM,
                                       musq[:, gs], ALU.mult, ALU.subtract)
        nc.scalar.activation(lv[:, gs], var[:, gs], ACTF.Ln, bias=eps_t[:])
        nc.scalar.activation(rstd[:, gs], lv[:, gs], ACTF.Exp, scale=-0.5)
        normed = []
        for i in range(g0, g0 + TCH):
            nt = nrm.tile([128, DM], BF16, tag="normed")
            nc.vector.tensor_scalar(nt[:], src_tiles[i][:], mu[:, i:i + 1],
                                    rstd[:, i:i + 1], ALU.subtract, ALU.mult)
            normed.append(nt)
        for f in range(DM // 128):
            pt = psT.tile([128, TCH * 128], BF16, tag="psT")
            for i in range(TCH):
                nc.tensor.transpose(pt[:, i * 128:(i + 1) * 128],
                                    normed[i][:, f * 128:(f + 1) * 128],
                                    ident_sb[:])
            nc.any.tensor_scalar(
                hfm[f][:, g0 * 128:(g0 + TCH) * 128], pt[:],
                g_t[f][:], b_t[f][:], ALU.mult, ALU.add)
    return hfm


def build_nc(L=L_FULL):
    n_tok = L // 128
    CH = min(512, L)
    n_ch = L // CH  # token chunks for matmul moving dim
    LH = L // 2     # scan half length
    n_res = LS // 128  # token tiles in this core's residual/FF slice

    nc = bacc.Bacc("TRN2", target_bir_lowering=False, debug=False,
                   num_devices=8)

    # The act-table chooser is first-match per function, so an Exp/Ln mix
    # alternates between the exp-only and ln-only sets (a ~1.3us table load
    # per switch). Steer both functions to the combined natural_log_exp set
    # (a real act_info set containing exp+ln) by hiding them in the others.
    from concourse.hw_specs import get_activation_tables
    try:
        tabs = get_activation_tables(nc.m.arch)
        for tname, tset in tabs.items():
            if tname != "natural_log_exp_and_others":
                tset.discard(ACTF.Exp)
                tset.discard(ACTF.Ln)
    except Exception:
        pass

    # ---- dram params ----
    def din(name, shape, dt=FP32):
        return nc.dram_tensor(name, shape, dt, kind="ExternalInput").ap()

    x_d = din("x", [L, DM], BF16)
    x_res_d = din("x_res", [LS, DM])               # this core's token slice
    # Small per-feature vectors are host-pretiled into packed 2-D tiles so
    # each loads with ONE DMA (Sync trigger time serializes otherwise).
    ln1_g = din("ln1_g", [128, DM // 128]); ln1_b = din("ln1_b", [128, DM // 128])
    ln2_g = din("ln2_g", [128, DM // 128]); ln2_b = din("ln2_b", [128, DM // 128])
    w_in_d = din("w_in", [DM, 2 * DIS], BF16)      # [xc cols | z cols]
    conv_w_d = din("conv_w", [DIS, DC])
    conv_b_d = din("conv_b", [128, 4])             # column d = channel tile d
    a_neg_d = din("a_neg", [DIS, NST])             # A = -exp(a_log) shard
    w_x_d = din("w_x", [DIS, RNK + 2 * NST], BF16)
    w_dt_d = din("w_dt", [RNK, DIS], BF16)
    b_dt_d = din("b_dt", [128, 4])
    d_skip_d = din("d_skip", [128, 4])
    w_out_d = din("w_out", [DIS, DM], BF16)
    w_ff1_d = din("w_ff1", [DM, 2 * FFI], BF16)    # FULL [a 4096 | g 4096]
    b_ff1_d = din("b_ff1", [2 * FFI, 1])           # FULL
    w_ff2_d = din("w_ff2", [FFI, DM], BF16)        # FULL
    ident_d = din("ident", [128, 128], BF16)

    out_d = nc.dram_tensor("out", [LS, DM], FP32, kind="ExternalOutput").ap()

    NCHK = 4               # FF column chunks
    FCH = FFI // NCHK      # 1024 a-cols + 1024 g-cols per chunk

    with tile.TileContext(nc) as tc, ExitStack() as octx:
        dram = octx.enter_context(tc.tile_pool(name="dram", bufs=1,
                                               space="DRAM"))
        const = octx.enter_context(tc.tile_pool(name="const", bufs=1))

        ident_sb = const.tile([128, 128], BF16, tag="ident")
        nc.sync.dma_start(out=ident_sb[:], in_=ident_d[:, :])

        # dram intermediates for collectives
        dbc_part = dram.tile([RNK + 2 * NST, L], BF16, tag="dbc_part")
        dbc_ar = dram.tile([RNK + 2 * NST, L], BF16, tag="dbc_ar")
        # ReduceScatter is chunked over 4 token blocks of 512 so each RS
        # overlaps the next block's out_proj matmuls. RS chunk k hands this
        # core rows [128*pos : 128*(pos+1)] of tokens [512k : 512(k+1)];
        # the host-side in/out maps are remapped to this interleaved
        # token ownership (LN2/FF are per-token, so compute is unaffected).
        m_part_c = [dram.tile([LS, DM], BF16, tag=f"m_part{k}",
                              name=f"m_part{k}") for k in range(4)]
        m_rs_c = [dram.tile([LS // 4, DM], BF16, tag=f"m_rs{k}",
                            name=f"m_rs{k}") for k in range(4)]

        groups = [[0, 1, 2, 3], [4, 5, 6, 7]]

        # Long-lived pools (phases 2-4) created first so shorter-lived
        # pools can pop in LIFO order before phase 5 reuses the space.
        pMain = octx.enter_context(ExitStack())
        wts = pMain.enter_context(tc.tile_pool(name="wts", bufs=1))
        sconst = pMain.enter_context(tc.tile_pool(name="sconst", bufs=1))
        act = pMain.enter_context(tc.tile_pool(name="act", bufs=1))
        zsp = pMain.enter_context(tc.tile_pool(name="zs", bufs=1))
        dtp = pMain.enter_context(tc.tile_pool(name="dtp", bufs=1))
        dtxp = pMain.enter_context(tc.tile_pool(name="dtx", bufs=1))
        dbcp = pMain.enter_context(tc.tile_pool(name="dbcp", bufs=1))

        # ================= Phase 1: LN1 -> h_fm =================
        p12 = pMain.enter_context(ExitStack())
        mm = p12.enter_context(tc.tile_pool(name="mm", bufs=4, space="PSUM"))
        hfm_pool = p12.enter_context(tc.tile_pool(name="hfm", bufs=8))
        with ExitStack() as p1, nc.named_scope("p1_ln1"):
            xload = p1.enter_context(tc.tile_pool(name="xload", bufs=6))
            xt_list = []
            for i in range(n_tok):
                xt = xload.tile([128, DM], BF16, tag="xt")
                nc.sync.dma_start(out=xt[:], in_=x_d[i * 128:(i + 1) * 128, :])
                xt_list.append(xt)
            hfm = _layer_norm_stage(nc, tc, p1, xt_list, n_tok, ident_sb,
                                    ln1_g, ln1_b, hfm_pool, L, "ln1")

        # ================= Phase 2: in_proj, conv, dbc, dt =================
        _sid2 = nc.enter_named_scope("p2_inproj", False)[0]
        w12 = p12.enter_context(tc.tile_pool(name="w12", bufs=1))
        # These early weight loads go out on the PE queue (idle at kernel
        # head) so their ~0.6us triggers don't serialize behind the x-tile
        # loads on the Sync queue.
        w_in_sb = []
        for k in range(8):
            t = w12.tile([128, 2 * DIS], BF16, tag=f"w_in{k}")
            nc.sync.dma_start(out=t[:], in_=w_in_d[k * 128:(k + 1) * 128, :])
            w_in_sb.append(t)
        wx_sb = []
        for k in range(4):
            t = w12.tile([128, RNK + 2 * NST], BF16, tag=f"wx{k}")
            nc.sync.dma_start(out=t[:], in_=w_x_d[k * 128:(k + 1) * 128, :])
            wx_sb.append(t)
        wdt_sb = w12.tile([RNK, DIS], BF16, tag="wdt")
        nc.sync.dma_start(out=wdt_sb[:], in_=w_dt_d[:, :])

        cw_sb, a_sb = [], []
        for d in range(4):
            r = slice(d * 128, (d + 1) * 128)
            t = sconst.tile([128, DC], FP32, tag=f"cw{d}")
            nc.sync.dma_start(out=t[:], in_=conv_w_d[r, :]); cw_sb.append(t)
            t = sconst.tile([128, NST], FP32, tag=f"a{d}")
            nc.sync.dma_start(out=t[:], in_=a_neg_d[r, :]); a_sb.append(t)
        cb_pack = sconst.tile([128, 4], FP32, tag="cb_pack")
        nc.sync.dma_start(out=cb_pack[:], in_=conv_b_d[:, :])
        cbn_pack = sconst.tile([128, 4], FP32, tag="cbn_pack")
        nc.vector.tensor_scalar_mul(cbn_pack[:], cb_pack[:], -1.0)
        bdt_pack = sconst.tile([128, 4], FP32, tag="bdt_pack")
        nc.sync.dma_start(out=bdt_pack[:], in_=b_dt_d[:, :])
        dskip_pack = sconst.tile([128, 4], FP32, tag="dskip_pack")
        nc.sync.dma_start(out=dskip_pack[:], in_=d_skip_d[:, :])
        cb_sb = [cb_pack[:, d:d + 1] for d in range(4)]
        cbn_sb = [cbn_pack[:, d:d + 1] for d in range(4)]
        bdt_sb = [bdt_pack[:, d:d + 1] for d in range(4)]
        dskip_sb = [dskip_pack[:, d:d + 1] for d in range(4)]

        xc_pad = []
        for d in range(4):
            t = act.tile([128, L + 3], BF16, tag=f"xcp{d}")
            nc.vector.memset(t[:, 0:3], 0.0)
            xc_pad.append(t)
        zraw = p12.enter_context(tc.tile_pool(name="zraw", bufs=1))
        z_sb = [zraw.tile([128, L], BF16, tag=f"z{d}", name=f"z{d}")
                for d in range(4)]

        # in_proj xc columns only (z columns are deferred into the
        # dbc-AllReduce window below)
        def inproj_f(f):
            for c in range(n_ch):
                ps = mm.tile([128, CH], FP32, tag="mm", name="ps")
                for k in range(8):
                    nc.tensor.matmul(
                        ps[:], w_in_sb[k][:, f * 128:(f + 1) * 128],
                        hfm[k][:, c * CH:(c + 1) * CH],
                        start=(k == 0), stop=(k == 7))
                if f < 4:
                    nc.any.tensor_copy(
                        xc_pad[f][:, 3 + c * CH: 3 + (c + 1) * CH], ps[:])
                else:
                    # Scalar-pinned: these run while DVE works the conv/silu
                    # chain on the critical path.
                    nc.scalar.copy(
                        z_sb[f - 4][:, c * CH:(c + 1) * CH], ps[:])

        for f in range(4):
            inproj_f(f)

        # conv + silu (writes silu'd xc back into xc_pad[:, 3:3+L]).
        # tensor_scalar runs in the DVE 2x/4x perf modes while
        # scalar_tensor_tensor is stuck at 1x, so the tap muls go through
        # separate tensor_scalar ops + 2x adds instead of an stt chain.
        # conv+silu run per L-half so the dbc matmuls (which consume 512-col
        # chunks) start on half-0 while half-1 is still in flight.
        cacc = p12.enter_context(tc.tile_pool(name="cacc", bufs=2))
        for d in range(4):
            acc = cacc.tile([128, L], BF16, tag="cacc")
            # Both halves' conv taps are emitted before either silu: the silu
            # overwrites xc_pad in place, and half-1's taps read the raw
            # boundary columns half-0's silu would clobber.
            for h in range(2):
                hs = slice(h * LH, (h + 1) * LH)
                nc.vector.tensor_scalar_mul(acc[:, hs],
                                            xc_pad[d][:, h * LH:h * LH + LH],
                                            cw_sb[d][:, 0:1])
                for j in range(1, DC):
                    tj = cacc.tile([128, LH], BF16, tag="ctap")
                    nc.vector.tensor_scalar_mul(
                        tj[:], xc_pad[d][:, j + h * LH:j + h * LH + LH],
                        cw_sb[d][:, j:j + 1])
                    if j == DC - 1:
                        # conv bias via a 4x-mode tensor_scalar add, then a
                        # 2x add (the stt equivalent runs at 1x)
                        nc.vector.tensor_scalar_add(tj[:], tj[:],
                                                    cb_sb[d][:])
                    nc.vector.tensor_add(acc[:, hs], acc[:, hs], tj[:])
            for h in range(2):
                hs = slice(h * LH, (h + 1) * LH)
                _silu_div(nc, cacc,
                          xc_pad[d][:, 3 + h * LH:3 + h * LH + LH],
                          acc[:, hs], "sil")

        # dbc partial + AllReduce
        nc.leave_named_scope("p2_inproj", _sid2, False)
        _sid2b = nc.enter_named_scope("p2b_dbc_ar", False)[0]
        dbc_sb = dbcp.tile([RNK + 2 * NST, L], BF16, tag="dbc")
        for c in range(n_ch):
            ps = mm.tile([RNK + 2 * NST, CH], FP32, tag="mm")
            for k in range(4):
                nc.tensor.matmul(ps[:], wx_sb[k][:],
                                 xc_pad[k][:, 3 + c * CH:3 + (c + 1) * CH],
                                 start=(k == 0), stop=(k == 3))
            nc.any.tensor_copy(dbc_sb[:, c * CH:(c + 1) * CH], ps[:])
        nc.sync.dma_start(out=dbc_part[:], in_=dbc_sb[:])
        if NOCOLL:
            nc.sync.dma_start(out=dbc_ar[:], in_=dbc_part[:])
        else:
            nc.gpsimd.collective_compute(
                "AllReduce", ALU.add, replica_groups=groups,
                ins=[dbc_part.opt()], outs=[dbc_ar.opt()])

        # ---- overlapped with the AllReduce: z in_proj, z-silu, prefetches
        for f in range(4, 8):
            inproj_f(f)
        zs_sb = []
        for d in range(4):
            zs = zsp.tile([128, L], BF16, tag=f"zs{d}")
            _silu_div(nc, cacc, zs[:], z_sb[d][:], "sil")
            zs_sb.append(zs)
        wout_sb = []
        for k in range(4):
            t = wts.tile([128, DM], BF16, tag=f"wout{k}")
            nc.sync.dma_start(out=t[:], in_=w_out_d[k * 128:(k + 1) * 128, :])
            wout_sb.append(t)

        nc.sync.dma_start(out=dbc_sb[:], in_=dbc_ar[:])
        dtlo = dbc_sb[0:RNK, :]
        nc.leave_named_scope("p2b_dbc_ar", _sid2b, False)
        _sid2c = nc.enter_named_scope("p2c_dt", False)[0]

        # dt = softplus(dt_lo @ w_dt + b_dt) = Ln(Exp(u + b_dt) + 1)
        # Half-0 chunks (0,1) are fully computed here so the scan can start;
        # half-1 chunks only run their matmul (PSUM dies with p12) and spill
        # the raw pre-activation, with softplus+dtx deferred into the scan's
        # half-0 window where the Act engine has slack.
        spscr = p12.enter_context(tc.tile_pool(name="spscr", bufs=3))
        dt_sb = [dtp.tile([128, L], BF16, tag=f"dt{d}", name=f"dt{d}")
                 for d in range(4)]
        dtx_sb = [dtxp.tile([128, L], BF16, tag=f"dtx{d}", name=f"dtx{d}")
                  for d in range(4)]
        uraw = [dtp.tile([128, LH], BF16, tag=f"u{d}", name=f"u{d}")
                for d in range(4)]
        for c in range(n_ch):
            cs = slice(c * CH, (c + 1) * CH)
            for d in range(4):
                ps = mm.tile([128, CH], FP32, tag="mm", name="ps")
                nc.tensor.matmul(ps[:], wdt_sb[:, d * 128:(d + 1) * 128],
                                 dtlo[:, cs].opt(),
                                 start=True, stop=True)
                if c < 2:
                    e = spscr.tile([128, CH], FP32, tag="sp_e")
                    nc.scalar.activation(e[:], ps[:], ACTF.Exp,
                                         bias=bdt_sb[d][:])
                    nc.scalar.activation(dt_sb[d][:, cs], e[:], ACTF.Ln,
                                         bias=1.0)
                    nc.vector.tensor_mul(
                        dtx_sb[d][:, cs], dt_sb[d][:, cs],
                        xc_pad[d][:, 3 + c * CH:3 + (c + 1) * CH])
                else:
                    nc.any.tensor_copy(
                        uraw[d][:, (c - 2) * CH:(c - 1) * CH], ps[:])
        p12.close()

        def emit_dt_half1(spool):
            for c in (2, 3):
                cs = slice(c * CH, (c + 1) * CH)
                us = slice((c - 2) * CH, (c - 1) * CH)
                for d in range(4):
                    e = spool.tile([128, CH], FP32, tag="sp_e1", name="e1")
                    nc.scalar.activation(e[:], uraw[d][:, us], ACTF.Exp,
                                         bias=bdt_sb[d][:])
                    nc.scalar.activation(dt_sb[d][:, cs], e[:], ACTF.Ln,
                                         bias=1.0)
                    nc.vector.tensor_mul(
                        dtx_sb[d][:, cs], dt_sb[d][:, cs],
                        xc_pad[d][:, 3 + c * CH:3 + (c + 1) * CH])

        nc.leave_named_scope("p2c_dt", _sid2c, False)
        _sid3 = nc.enter_named_scope("p3_scan", False)[0]

        # ========= Phase 3+4: scan + gate + out_proj + RS, fused =========
        # Four L-quarters instead of two halves: the y PSUM accumulation for
        # a quarter needs only 4 banks (4 d-tiles x [128,512] fp32), leaving
        # 2 banks for out_proj, so each quarter's out_proj matmuls and its
        # ReduceScatter chunk run inside the next quarter's scan shadow
        # instead of serially after the whole scan.
        LQ = L // 4
        p34 = pMain.enter_context(ExitStack())
        gp = p34.enter_context(tc.tile_pool(name="gated", bufs=1))
        ypsum = p34.enter_context(tc.tile_pool(name="ypsum", bufs=1,
                                               space="PSUM"))
        mmo = p34.enter_context(tc.tile_pool(name="mmo", bufs=2, space="PSUM"))
        mp_pool = p34.enter_context(tc.tile_pool(name="mp", bufs=3))
        sc = p34.enter_context(tc.tile_pool(name="scan", bufs=2))
        bcp = p34.enter_context(tc.tile_pool(name="bcast", bufs=2))
        hlast = [sconst.tile([128, NST], FP32, tag=f"hl{d}", name=f"hl{d}")
                 for d in range(4)]
        gated_sb = [gp.tile([128, L], BF16, tag=f"g{d}", name=f"g{d}")
                    for d in range(4)]

        for q in range(4):
            off = q * LQ
            y_ps = [ypsum.tile([128, LQ], FP32, tag=f"yps{d}", name=f"yps{d}")
                    for d in range(4)]
            for n0 in range(0, NST, 8):
                # States are processed in quads: the dB and hC multiplies for
                # (n0..n0+3) fuse into single [128, 4, LQ] DVE ops (2x perf
                # mode; per-instruction overhead amortizes over 2048 cols),
                # with dtx broadcast stride-0 across the quad dim. Scans stay
                # per-n (serial recurrence).
                bcB = bcp.tile([128, 8, LQ], BF16, tag="bcB")
                bcC = bcp.tile([128, 8, LQ], BF16, tag="bcC")
                for j in range(8):
                    nc.sync.dma_start(
                        out=bcB[:, j, :],
                        in_=dbc_ar[RNK + n0 + j:RNK + n0 + j + 1,
                                   off:off + LQ].broadcast_to([128, LQ]))
                    nc.sync.dma_start(
                        out=bcC[:, j, :],
                        in_=dbc_ar[RNK + NST + n0 + j:RNK + NST + n0 + j + 1,
                                   off:off + LQ].broadcast_to([128, LQ]))
                for d in range(4):
                    dA = sc.tile([128, 8, LQ], BF16, tag="dA")
                    for j in range(8):
                        nc.scalar.activation(
                            dA[:, j, :], dt_sb[d][:, off:off + LQ],
                            ACTF.Exp, scale=a_sb[d][:, n0 + j:n0 + j + 1])
                    dB = sc.tile([128, 8, LQ], BF16, tag="dB")
                    nc.vector.tensor_mul(
                        dB[:, :, :],
                        dtx_sb[d][:, off:off + LQ].unsqueeze(1)
                        .broadcast_to([128, 8, LQ]),
                        bcB[:, :, :])
                    for j in range(8):
                        n = n0 + j
                        # h state scan (in-place over dB), fp32 internal state
                        init = 0.0 if q == 0 else hlast[d][:, n:n + 1]
                        nc.vector.tensor_tensor_scan(
                            dB[:, j, :], dA[:, j, :], dB[:, j, :], init,
                            ALU.mult, ALU.add)
                        if q < 3:
                            nc.scalar.activation(
                                hlast[d][:, n:n + 1],
                                dB[:, j, LQ - 1:LQ], ACTF.Identity)
                    # hC into dA tile (reuse), then accumulate on PE.
                    # All hC muls stay on DVE: a concurrent GpSimd tensor op
                    # locks the shared SBUF port pair and quadruples the DVE
                    # op latency (602ns -> 2460ns measured), so offloading to
                    # Pool is strictly counterproductive.
                    nc.vector.tensor_mul(dA[:, :, :], dB[:, :, :], bcC[:, :, :])
                    for j in range(8):
                        nc.tensor.matmul(
                            y_ps[d][:], ident_sb[:], dA[:, j, :],
                            start=(n0 + j == 0), stop=(n0 + j == NST - 1))
            if q == 1:
                emit_dt_half1(sc)
            # gate: gated = (y + d_skip*xc) * silu(z)
            for d in range(4):
                tmp = sc.tile([128, LQ], FP32, tag="gtmp")
                nc.vector.scalar_tensor_tensor(
                    tmp[:], xc_pad[d][:, 3 + off:3 + off + LQ],
                    dskip_sb[d][:], y_ps[d][:], ALU.mult, ALU.add)
                nc.vector.tensor_mul(gated_sb[d][:, off:off + LQ], tmp[:],
                                     zs_sb[d][:, off:off + LQ])
            # out_proj for this quarter's 4 token tiles + its RS chunk
            for i in range(4 * q, 4 * q + 4):
                mp = mp_pool.tile([128, DM], BF16, tag="mp")
                for nchk in range(2):
                    ps = mmo.tile([128, 512], FP32, tag="mmo")
                    for k in range(4):
                        nc.tensor.matmul(
                            ps[:], gated_sb[k][:, i * 128:(i + 1) * 128],
                            wout_sb[k][:, nchk * 512:(nchk + 1) * 512],
                            start=(k == 0), stop=(k == 3))
                    # Pinned to Scalar: DVE is saturated during the scan.
                    nc.scalar.copy(mp[:, nchk * 512:(nchk + 1) * 512], ps[:])
                nc.sync.dma_start(
                    out=m_part_c[q][(i % 4) * 128:(i % 4 + 1) * 128, :],
                    in_=mp[:])
            if NOCOLL:
                nc.sync.dma_start(out=m_rs_c[q][:],
                                  in_=m_part_c[q][0:LS // 4, :])
            else:
                nc.gpsimd.collective_compute(
                    "ReduceScatter", ALU.add, replica_groups=groups,
                    ins=[m_part_c[q].opt()], outs=[m_rs_c[q].opt()])

        nc.leave_named_scope("p3_scan", _sid3, False)
        _sid4 = nc.enter_named_scope("p4_outproj", False)[0]
        p34.close()
        pMain.close()

        # ---- overlapped with the ReduceScatter: FF chunk-0 weight loads ----
        wf = octx.enter_context(tc.tile_pool(name="wf", bufs=2))
        wf2 = octx.enter_context(tc.tile_pool(name="wf2", bufs=2))
        bfp = octx.enter_context(tc.tile_pool(name="bfp", bufs=1))

        def load_ff_chunk(c):
            # fp8 weights in 3-D [128, k_subtile, cols] layout so matmuls can
            # take [:, 2k:2k+2, :] DoubleRow slices (K=256 per matmul).
            a0, g0 = c * FCH, FFI + c * FCH
            w1a = wf.tile([128, 8, FCH], BF16, tag="w1a", name="w1a")
            w1g = wf.tile([128, 8, FCH], BF16, tag="w1g", name="w1g")
            for k in range(8):
                nc.sync.dma_start(
                    out=w1a[:, k, :],
                    in_=w_ff1_d[k * 128:(k + 1) * 128, a0:a0 + FCH])
                nc.sync.dma_start(
                    out=w1g[:, k, :],
                    in_=w_ff1_d[k * 128:(k + 1) * 128, g0:g0 + FCH])
            w2 = wf2.tile([128, 8, DM], BF16, tag="w2", name="w2")
            for k in range(8):
                nc.sync.dma_start(
                    out=w2[:, k, :],
                    in_=w_ff2_d[c * FCH + k * 128:c * FCH + (k + 1) * 128, :])
            ba = [bfp.tile([128, 1], FP32, tag=f"ba{c}_{j}", name=f"ba{c}_{j}")
                  for j in range(8)]
            bg = [bfp.tile([128, 1], FP32, tag=f"bg{c}_{j}", name=f"bg{c}_{j}")
                  for j in range(8)]
            for j in range(8):
                nc.sync.dma_start(
                    out=ba[j][:],
                    in_=b_ff1_d[a0 + j * 128:a0 + (j + 1) * 128, :])
                nc.sync.dma_start(
                    out=bg[j][:],
                    in_=b_ff1_d[g0 + j * 128:g0 + (j + 1) * 128, :])
            return w1a, w1g, w2, ba, bg

        ff_chunk = load_ff_chunk(0)

        # ================= Phase 5: residual + LN2 (own L/4 slice) ==========
        # h2 is written straight to fp8 in 3-D [128, k, LS] layout for the
        # DoubleRow ff1 matmuls.
        h2fm_pool = octx.enter_context(tc.tile_pool(name="h2fm", bufs=1))
        h2t = h2fm_pool.tile([128, 8, LS], BF16, tag="h2t", name="h2t")
        h2_views = [h2t[:, f, :] for f in range(8)]
        x2p = octx.enter_context(tc.tile_pool(name="x2", bufs=1))
        xres_p = octx.enter_context(tc.tile_pool(name="xres", bufs=1))
        xres_sb = []
        for i in range(n_res):
            t = xres_p.tile([128, DM], FP32, tag=f"xres{i}", name=f"xres{i}")
            nc.sync.dma_start(out=t[:], in_=x_res_d[i * 128:(i + 1) * 128, :])
            xres_sb.append(t)
        x2_list = []
        with ExitStack() as p5, nc.named_scope("p5_ln2"):
            ld = p5.enter_context(tc.tile_pool(name="ld5", bufs=3))
            for i in range(n_res):
                mt = ld.tile([128, DM], BF16, tag="mr")
                nc.sync.dma_start(out=mt[:], in_=m_rs_c[i][:, :])
                x2 = x2p.tile([128, DM], FP32, tag=f"x2_{i}")
                nc.vector.tensor_add(x2[:], xres_sb[i][:], mt[:])
                x2_list.append(x2)
            # TCH=2: group {0,1} only needs RS chunks 0-1, so most of LN2
            # runs inside the scan shadow; only the tile-3 group trails RS_3.
            h2fm = _layer_norm_stage(nc, tc, p5, x2_list, n_res, ident_sb,
                                     ln2_g, ln2_b, h2fm_pool, LS, "ln2",
                                     out_views=h2_views, TCH=2)

        # ================= Phase 6: FF (full width, 4 chunks) ===============
        with ExitStack() as p6, nc.named_scope("p6_ff"):
            mma6 = p6.enter_context(tc.tile_pool(name="mma6", bufs=2,
                                                 space="PSUM"))
            mm6 = p6.enter_context(tc.tile_pool(name="mm6", bufs=2,
                                                space="PSUM"))
            pso_pool = p6.enter_context(tc.tile_pool(name="pso", bufs=2,
                                                     space="PSUM"))
            agp = p6.enter_context(tc.tile_pool(name="ag", bufs=2))
            tmp6 = p6.enter_context(tc.tile_pool(name="tmp6", bufs=4))
            acc_p = p6.enter_context(tc.tile_pool(name="ffacc", bufs=1))
            outp = p6.enter_context(tc.tile_pool(name="outp", bufs=2))
            out_acc = [acc_p.tile([128, DM], FP32, tag=f"oacc{i}", name=f"oacc{i}")
                       for i in range(n_res)]

            for c in range(NCHK):
                w1a, w1g, w2, ba, bg = ff_chunk
                if c + 1 < NCHK:
                    next_chunk = load_ff_chunk(c + 1)
                ag_t = agp.tile([128, 8, LS], BF16, tag="ag")
                for sub in range(FCH // 128):
                    psA = mma6.tile([128, LS], FP32, tag="mma6")
                    psG = mm6.tile([128, LS], FP32, tag="mm6")
                    for k in range(8):
                        nc.tensor.matmul(
                            psA[:], w1a[:, k, sub * 128:(sub + 1) * 128],
                            h2t[:, k, :], start=(k == 0), stop=(k == 7))
                    for k in range(8):
                        nc.tensor.matmul(
                            psG[:], w1g[:, k, sub * 128:(sub + 1) * 128],
                            h2t[:, k, :], start=(k == 0), stop=(k == 7))
                    aa = tmp6.tile([128, LS], BF16, tag="aa")
                    nc.scalar.activation(aa[:], psA[:], ACTF.Identity,
                                         bias=ba[sub][:])
                    gg = tmp6.tile([128, LS], BF16, tag="gg")
                    nc.scalar.activation(gg[:], psG[:], ACTF.Gelu_apprx_tanh,
                                         bias=bg[sub][:])
                    nc.vector.tensor_mul(ag_t[:, sub, :], aa[:], gg[:])
                # ff2: token-major, accumulate chunks in SBUF fp32
                for tt in range(n_res):
                    pso = pso_pool.tile([128, DM], FP32, tag="pso")
                    for q in range(DM // 512):
                        for k in range(8):
                            nc.tensor.matmul(
                                pso[:, q * 512:(q + 1) * 512],
                                ag_t[:, k, tt * 128:(tt + 1) * 128],
                                w2[:, k, q * 512:(q + 1) * 512],
                                start=(k == 0), stop=(k == 7))
                    if c == 0:
                        nc.any.tensor_copy(out_acc[tt][:], pso[:])
                    else:
                        nc.vector.tensor_add(out_acc[tt][:], out_acc[tt][:],
                                             pso[:])
                    if c == NCHK - 1:
                        # final out = x2 + ff, inlined per tile so each
                        # output streams out while the remaining tiles'
                        # ff2 matmuls are still running
                        ot = outp.tile([128, DM], FP32, tag="ot")
                        nc.vector.tensor_add(ot[:], x2_list[tt][:],
                                             out_acc[tt][:])
                        nc.sync.dma_start(
                            out=out_d[tt * 128:(tt + 1) * 128, :], in_=ot[:])
                if c + 1 < NCHK:
                    ff_chunk = next_chunk
        nc.leave_named_scope("p4_outproj", _sid4, False)
    nc.compile()
    return nc


_NC_CACHE = {}


def _get_nc(L=L_FULL):
    if L not in _NC_CACHE:
        _NC_CACHE[L] = build_nc(L)
    return _NC_CACHE[L]


def make_in_maps(x, ln1_g, ln1_b, w_in, conv_w, conv_b, w_x, w_dt, b_dt,
                 a_log, d_skip, w_out, ln2_g, ln2_b, w_ff1, b_ff1, w_ff2,
                 b_ff2):
    x = np.asarray(x, np.float32)
    f32 = lambda a: np.ascontiguousarray(np.asarray(a, np.float32))
    bf = lambda a: np.ascontiguousarray(np.asarray(a, np.float32)).astype(NPBF16)
    # pack a per-feature vector [n*128] -> [128, n] (column j = slice j)
    pack = lambda a, n: f32(np.asarray(a, np.float32).reshape(n, 128).T)
    ident = np.eye(128, dtype=np.float32).astype(NPBF16)
    a_neg = -np.exp(np.asarray(a_log, np.float32))
    w_ff1_b = bf(w_ff1)
    w_ff2_b = bf(w_ff2)
    b_ff1_c = f32(b_ff1).reshape(2 * FFI, 1)
    in_maps = []
    for c in range(8):
        b, s = c // 4, c % 4
        ds = slice(s * DIS, (s + 1) * DIS)
        # Chunked-RS token ownership: this core owns rows
        # [512k + 128s : 512k + 128(s+1)] for k = 0..3.
        own = np.concatenate(
            [x[b][512 * k + 128 * s: 512 * k + 128 * (s + 1)]
             for k in range(4)], axis=0)
        in_maps.append(dict(
            x=bf(x[b]),
            x_res=f32(own),
            ln1_g=pack(ln1_g, 8), ln1_b=pack(ln1_b, 8),
            ln2_g=pack(ln2_g, 8), ln2_b=pack(ln2_b, 8),
            w_in=bf(np.concatenate(
                [w_in[:, s * DIS:(s + 1) * DIS],
                 w_in[:, DI + s * DIS:DI + (s + 1) * DIS]], axis=1)),
            conv_w=f32(conv_w[ds]), conv_b=pack(np.asarray(conv_b)[ds], 4),
            a_neg=f32(a_neg[ds]),
            w_x=bf(w_x[ds]), w_dt=bf(w_dt[:, ds]),
            b_dt=pack(np.asarray(b_dt)[ds], 4),
            d_skip=pack(np.asarray(d_skip)[ds], 4),
            w_out=bf(w_out[ds]),
            w_ff1=w_ff1_b, b_ff1=b_ff1_c, w_ff2=w_ff2_b,
            ident=ident,
        ))
    return in_maps


def combine_outputs(results, b_ff2, L=L_FULL):
    out = np.zeros((B_FULL, L, DM), np.float32)
    bff2 = np.asarray(b_ff2, np.float32)
    for b in range(B_FULL):
        for s in range(4):
            res = results[4 * b + s]["out"].astype(np.float32) + bff2[None, :]
            for k in range(4):
                out[b, 512 * k + 128 * s: 512 * k + 128 * (s + 1)] = (
                    res[128 * k: 128 * (k + 1)])
    return out


def kernel(**inputs):
    nc = _get_nc(L_FULL)
    in_maps = make_in_maps(
        inputs["x"], inputs["ln1_g"], inputs["ln1_b"], inputs["w_in"],
        inputs["conv_w"], inputs["conv_b"], inputs["w_x"], inputs["w_dt"],
        inputs["b_dt"], inputs["a_log"], inputs["d_skip"], inputs["w_out"],
        inputs["ln2_g"], inputs["ln2_b"], inputs["w_ff1"], inputs["b_ff1"],
        inputs["w_ff2"], inputs["b_ff2"])
    res = run_bass_kernel_spmd(nc, in_maps, core_ids=list(range(8)))
    return combine_outputs(res.results, inputs["b_ff2"], L_FULL)

